# revision 28
# baseline (speedup 1.0000x reference)
"""Trainium2 Bass kernel for a single-head transformer block.

Reference computation (B=4, S=4096, D=1024, fp32):
    h   = rmsnorm(x) * g
    qkv = h @ w_qkv + b_qkv ;  q,k,v = split(qkv)
    q,k = ternary_rope(q), ternary_rope(k)      (cos/sin rounded to {-1,0,1})
    p   = softmax(q@k.T / sqrt(D) * ln3)        (base-3 softmax)
    out = (p @ v) @ w_proj + b_proj + x

Sharding: 8 cores, 2 per batch. Each core computes K/V for its full batch
(4096 keys) and attention for its 2048 query rows. Per-core inputs are
reordered so the core's own query rows come first (attention over keys is
permutation invariant); rope tables are passed per-core in the same order.

All big matmuls run in fp8e4 with DoubleRow perf mode (2 k-slabs of 128 per
instruction, ~1.4x bf16 throughput). K^T, Q^T and V are kept resident in
SBUF in fp8 (no DRAM roundtrip). Weights are pre-scaled into fp8 on the
host (wqkv*64, wproj*8); the scales are folded back in the PSUM-evacuation
copies and the softmax-normalization multiply. fp32 is kept only for PSUM
accumulation, softmax denominators and the residual path.
"""

import numpy as np
import ml_dtypes

import concourse.bass as bass
import concourse.tile as tile
from concourse import mybir
from concourse.bass_utils import run_bass_kernel_spmd
from concourse.masks import make_identity

BF16 = mybir.dt.bfloat16
F32 = mybir.dt.float32
F8 = mybir.dt.float8e4
F8NP = ml_dtypes.float8_e4m3   # TRN fp8e4: bias 7, max normal 240

B, S, D = 4, 4096, 1024
P = 128
HALF = S // 2          # 2048 query rows per core
N_CORES = 8
RCH = 512              # row chunk for the qkv phase
ND = D // P            # 8 d-slabs
NSUB = RCH // P        # 4

EPS = 1e-6
LN3 = 1.0986122886681098
ROPE_BASE = 10000.0

WS = 64.0              # host scale on w_qkv (fp8 range use); undone at PSUM copy
WPS = 8.0              # host scale on w_proj
OS = 1.0 / WPS         # scale o into fp8; WPS*OS==1 so no extra compensation

DR = mybir.MatmulPerfMode.DoubleRow

LAST_RESULT = None     # BassKernelResults of the most recent run (for test.py)


def _split_multiwait(nc, max_waits=1):
    """Walrus in this build rejects instructions carrying many sem waits
    (the Tile end-of-kernel drain has one per engine/queue). Hoist excess
    waits onto single-wait NoOps just before the offending instruction."""
    for fn in nc.m.functions:
        for blk in fn.blocks:
            insts = list(blk.instructions)
            out, changed = [], False
            for ins in insts:
                si = ins.sync_info
                waits = list(si.on_wait) if si is not None and si.on_wait else []
                if len(waits) > max_waits:
                    changed = True
                    for j, w in enumerate(waits[:-max_waits]):
                        out.append(mybir.InstNoOp(
                            name=f"{ins.name}-sw{j}",
                            engine=ins.engine,
                            sync_info=mybir.SyncInfo(on_wait=[w], on_update=[]),
                            bass_nofuse=True,
                        ))
                    ins.sync_info = mybir.SyncInfo(
                        on_wait=waits[-max_waits:],
                        on_update=list(si.on_update) if si.on_update else [])
                out.append(ins)
            if changed:
                blk.instructions = out


def _ternary_tables(S=S):
    """Ternary rope cos/sin half-tables, transposed: [D/2, S] float32."""
    half = D // 2
    inv_freq = (1.0 / (ROPE_BASE ** (np.arange(half, dtype=np.float32) / half))
                ).astype(np.float32)
    ang = np.arange(S, dtype=np.float32)[:, None] * inv_freq[None, :]  # [S, half]
    cos = np.round(np.cos(ang)).astype(np.float32)
    sin = np.round(np.sin(ang)).astype(np.float32)
    return cos.T.copy(), sin.T.copy()  # [half, S]


def _to_f8(a):
    return np.clip(np.asarray(a, np.float32), -240.0, 240.0).astype(F8NP)


def _prepare_in_maps(x, g_norm, w_qkv, b_qkv, w_proj, b_proj, S=S):
    HALF = S // 2
    cos_h, sin_h = _ternary_tables(S)
    wqkv_f8 = np.ascontiguousarray(_to_f8(g_norm[:, None] * w_qkv * WS))
    wp_f8 = np.ascontiguousarray(_to_f8(w_proj * WPS))
    in_maps = []
    for c in range(N_CORES):
        b, h = c // 2, c % 2
        own = slice(h * HALF, (h + 1) * HALF)
        other = slice((1 - h) * HALF, (2 - h) * HALF)
        perm = np.concatenate([np.arange(own.start, own.stop),
                               np.arange(other.start, other.stop)])
        xb = x[b]
        in_maps.append({
            "x_t": np.ascontiguousarray(xb[perm].T).astype(ml_dtypes.bfloat16),
            "res": np.ascontiguousarray(xb[own] + b_proj[None, :]),
            "wqkv": wqkv_f8,
            "wp": wp_f8,
            "bqkv": b_qkv,
            "cos_t": np.ascontiguousarray(cos_h[:, perm]).astype(ml_dtypes.bfloat16),
            "sin_t": np.ascontiguousarray(sin_h[:, perm]).astype(ml_dtypes.bfloat16),
        })
    return in_maps


def _build(has_bqkv: bool, S=S, ph12=True, ph3=True, split=True):
    HALF = S // 2
    NKT = S // P
    nc = bass.Bass("TRN2", target_bir_lowering=False, debug=False,
                   num_devices=N_CORES)

    x_t = nc.dram_tensor("x_t", [D, S], BF16, kind="ExternalInput").ap()
    res_d = nc.dram_tensor("res", [HALF, D], F32, kind="ExternalInput").ap()
    wqkv_d = nc.dram_tensor("wqkv", [D, 3 * D], F8, kind="ExternalInput").ap()
    wp_d = nc.dram_tensor("wp", [D, D], F8, kind="ExternalInput").ap()
    bqkv_d = nc.dram_tensor("bqkv", [3 * D], F32, kind="ExternalInput").ap()
    cos_d = nc.dram_tensor("cos_t", [D // 2, S], BF16, kind="ExternalInput").ap()
    sin_d = nc.dram_tensor("sin_t", [D // 2, S], BF16, kind="ExternalInput").ap()
    out_d = nc.dram_tensor("out", [HALF, D], F32, kind="ExternalOutput").ap()

    xt_r = x_t.rearrange("(o p) s -> p o s", p=P)          # [128, 8, S]
    wqkv_r = wqkv_d.rearrange("(o p) n -> p o n", p=P)     # [128, 8, 3072]
    wp_r = wp_d.rearrange("(o p) n -> p o n", p=P)         # [128, 8, 1024]
    bqkv_r = bqkv_d.rearrange("(o p) -> p o", p=P)         # [128, 24]
    cos_r = cos_d.rearrange("(o p) s -> p o s", p=P)       # [128, 4, 4096]
    sin_r = sin_d.rearrange("(o p) s -> p o s", p=P)

    with tile.TileContext(nc) as tc:
        with (
            tc.tile_pool(name="singles", bufs=1) as singles,
        ):
            ident = singles.tile([P, P], F32)
            make_identity(nc, ident)
            ones_bf = singles.tile([P, 1], F8)
            nc.vector.memset(ones_bf, 1.0)
            onesc = singles.tile([1, P], BF16)
            nc.vector.memset(onesc, 1.0)
            eps_sb = singles.tile([P, 1], F32)
            nc.vector.memset(eps_sb, EPS)
            wp_sb = singles.tile([P, ND, D], F8)
            bqkv_sb = singles.tile([P, 24], F32)
            if has_bqkv:
                nc.sync.dma_start(bqkv_sb, bqkv_r)

            kt_s = singles.tile([P, ND, S], F8)        # rope'd K^T (SBUF)
            qt_s = singles.tile([P, ND, HALF], F8)     # rope'd Q^T (SBUF)
            v_s = singles.tile([P, NKT, D], F8)        # V, keys on partitions

            # ---------------- Phase 1+2: rmsnorm + QKV + rope ----------------
            if ph12:
                _phase12(nc, tc, S, has_bqkv, xt_r, wqkv_r, cos_r, sin_r,
                         bqkv_d, bqkv_sb, ones_bf, onesc, eps_sb,
                         kt_s, qt_s, v_s)
            if ph3:
                _phase3(nc, tc, S, wp_sb, wp_r, ident, res_d, out_d,
                        kt_s, qt_s, v_s)

    if split:
        _split_multiwait(nc)
    return nc


def _phase12(nc, tc, S, has_bqkv, xt_r, wqkv_r, cos_r, sin_r, bqkv_d, bqkv_sb,
             ones_bf, onesc, eps_sb, kt_s, qt_s, v_s):
    N_RCH = S // RCH
    N_QCH = max((S // 2) // RCH, 1)
    with (
        tc.tile_pool(name="wq", bufs=1) as wq_pool,
        tc.tile_pool(name="p12", bufs=2) as p12,
        tc.tile_pool(name="tmp12", bufs=1) as tmp12,
        tc.tile_pool(name="st", bufs=2) as st,
        tc.tile_pool(name="ps12", bufs=4, space="PSUM") as ps12,
        tc.tile_pool(name="psms", bufs=2, space="PSUM") as psms,
    ):
        # --- software-pipelined rmsnorm chain: stage A (loads + squares) and
        # stage B (mean -> rsqrt -> broadcast -> hT) are issued for chunk r+1
        # while chunk r's QKV matmuls keep the PE busy.
        def chain_a(r):
            rows = slice(r * RCH, (r + 1) * RCH)
            xT = p12.tile([P, ND, RCH], BF16, tag="xT", name=f"xT{r}")
            nc.sync.dma_start(xT, xt_r[:, :, rows])
            cos_c = p12.tile([P, 4, RCH], BF16, tag="cos", name=f"cos{r}")
            nc.sync.dma_start(cos_c, cos_r[:, :, rows])
            sin_c = p12.tile([P, 4, RCH], BF16, tag="sin", name=f"sin{r}")
            nc.sync.dma_start(sin_c, sin_r[:, :, rows])
            # fp8 squares are plenty: quantization noise averages out in
            # the 1024-term mean
            sq = p12.tile([P, ND, RCH], F8, tag="sq", name=f"sq{r}")
            nc.scalar.activation(sq, xT, mybir.ActivationFunctionType.Square)
            return xT, sq, cos_c, sin_c

        def chain_b1(r, sq):
            ps_ms = psms.tile([1, RCH], F32, tag="ms", name=f"ms{r}")
            for di in range(ND):
                nc.tensor.matmul(ps_ms, ones_bf, sq[:, di, :],
                                 start=(di == 0), stop=(di == ND - 1))
            sr = st.tile([1, RCH], F32, tag="sr", name=f"sr{r}")
            nc.scalar.activation(sr, ps_ms,
                                 mybir.ActivationFunctionType.Sqrt,
                                 bias=eps_sb[0:1, :], scale=1.0 / D)
            rf = st.tile([1, RCH], F32, tag="rf", name=f"rf{r}")
            nc.vector.reciprocal(rf, sr)
            rb = st.tile([1, RCH], BF16, tag="rb", name=f"rb{r}")
            nc.vector.tensor_copy(rb, rf)
            return rb

        def chain_b2(r, xT, rb):
            # broadcast r across partitions via a K=1 ones-matmul
            psr = psms.tile([P, RCH], F32, tag="psr", name=f"psr{r}")
            nc.tensor.matmul(psr, onesc, rb, start=True, stop=True)
            rep = p12.tile([P, RCH], BF16, tag="rep", name=f"rep{r}")
            nc.scalar.copy(rep, psr)
            hT = p12.tile([P, ND, RCH], F8, tag="hT", name=f"hT{r}")
            for di in range(ND):
                nc.vector.tensor_tensor(hT[:, di, :], xT[:, di, :], rep,
                                        mybir.AluOpType.mult)
            return hT

        def qk_mms(r, base, hT, t_qk):
            for do in range(ND):
                ps = ps12.tile([P, RCH], F32, tag="ps12")
                for pi in range(4):
                    nc.tensor.matmul(
                        ps,
                        wq_pairs[pi][:, :, base + do * P: base + (do + 1) * P],
                        hT[:, 2 * pi:2 * pi + 2, :],
                        start=(pi == 0), stop=(pi == 3), perf_mode=DR)
                if has_bqkv:
                    nc.scalar.activation(
                        t_qk[:, do, :], ps,
                        mybir.ActivationFunctionType.Identity,
                        bias=bqkv_sb[:, base // P + do: base // P + do + 1],
                        scale=1.0 / WS)
                else:
                    nc.scalar.mul(t_qk[:, do, :], ps, 1.0 / WS)

        def rope(r, t_qk, cos_c, sin_c, dst):
            rows = slice(r * RCH, (r + 1) * RCH)
            m1 = tmp12.tile([P, 4, RCH], BF16, tag="m1")
            nc.vector.tensor_tensor(m1, t_qk[:, 0:4, :], cos_c,
                                    mybir.AluOpType.mult)
            m2 = tmp12.tile([P, 4, RCH], BF16, tag="m2")
            nc.vector.tensor_tensor(m2, t_qk[:, 4:8, :], sin_c,
                                    mybir.AluOpType.mult)
            nc.vector.tensor_tensor(dst[:, 0:4, rows], m1, m2,
                                    mybir.AluOpType.subtract)
            m3 = tmp12.tile([P, 4, RCH], BF16, tag="m3")
            nc.vector.tensor_tensor(m3, t_qk[:, 4:8, :], cos_c,
                                    mybir.AluOpType.mult)
            m4 = tmp12.tile([P, 4, RCH], BF16, tag="m4")
            nc.vector.tensor_tensor(m4, t_qk[:, 0:4, :], sin_c,
                                    mybir.AluOpType.mult)
            nc.vector.tensor_tensor(dst[:, 4:8, rows], m3, m4,
                                    mybir.AluOpType.add)

        def v_mms(r, hT):
            for sub in range(NSUB):
                for no in range(D // 512):
                    ps = ps12.tile([P, RCH], F32, tag="ps12")
                    for pi in range(4):
                        nc.tensor.matmul(
                            ps,
                            hT[:, 2 * pi:2 * pi + 2, sub * P:(sub + 1) * P],
                            wq_pairs[pi][:, :, 2 * D + no * 512:
                                         2 * D + (no + 1) * 512],
                            start=(pi == 0), stop=(pi == 3), perf_mode=DR)
                    vdst = v_s[:, r * NSUB + sub, no * 512:(no + 1) * 512]
                    if has_bqkv:
                        vt = tmp12.tile([P, 512], BF16, tag="vt")
                        nc.scalar.mul(vt, ps, 1.0 / WS)
                        nc.vector.tensor_tensor(
                            vdst, vt,
                            bass.AP(tensor=bqkv_d.tensor,
                                    offset=bqkv_d.offset + 2 * D + no * 512,
                                    ap=[[0, P], [1, 512]]),
                            mybir.AluOpType.add)
                    else:
                        nc.scalar.mul(vdst, ps, 1.0 / WS)

        # chunk 0 inputs first so its norm chain starts before the big
        # weight loads hog the DMA queue
        a0 = chain_a(0)
        wq_pairs = [wq_pool.tile([P, 2, 3 * D], F8, tag=f"wq{pi}",
                                 name=f"wq{pi}") for pi in range(ND // 2)]
        # K columns first: the first matmul group of chunk 0 needs only them
        for c0 in (D, 0, 2 * D):
            for pi in range(ND // 2):
                nc.sync.dma_start(wq_pairs[pi][:, :, c0:c0 + D],
                                  wqkv_r[:, 2 * pi:2 * pi + 2, c0:c0 + D])
        rb0 = chain_b1(0, a0[1])
        hT0 = chain_b2(0, a0[0], rb0)
        cur = (hT0, a0[2], a0[3])

        for r in range(N_RCH):
            hT, cos_c, sin_c = cur
            nxt_a = chain_a(r + 1) if r + 1 < N_RCH else None
            # K
            t_k = p12.tile([P, ND, RCH], BF16, tag="tqk", name=f"tk{r}")
            qk_mms(r, D, hT, t_k)
            # r+1's norm math goes ahead of chunk r's ropes on the DVE
            # queue so hT(r+1) is ready before the PE needs it
            if nxt_a is not None:
                nxt_rb = chain_b1(r + 1, nxt_a[1])
                nxt_hT = chain_b2(r + 1, nxt_a[0], nxt_rb)
                cur = (nxt_hT, nxt_a[2], nxt_a[3])
            rope(r, t_k, cos_c, sin_c, kt_s)
            # Q (first half of rows only)
            if r < N_QCH:
                t_q = p12.tile([P, ND, RCH], BF16, tag="tqk", name=f"tq{r}")
                qk_mms(r, 0, hT, t_q)
                rope(r, t_q, cos_c, sin_c, qt_s)
            v_mms(r, hT)


def _phase3(nc, tc, S, wp_sb, wp_r, ident, res_d, out_d, kt_s, qt_s, v_s):
    import os
    lvl = int(os.environ.get("K3LVL", "4"))
    nc.sync.dma_start(wp_sb, wp_r)
    N_QCH = max((S // 2) // RCH, 1)
    NKT = S // P
    # ---------------- Phase 3: attention + proj + residual -----------
    with (
        tc.tile_pool(name="p3", bufs=2) as p3,
        tc.tile_pool(name="outp", bufs=4) as outp,
        tc.tile_pool(name="rcp", bufs=2) as rcp,
        tc.tile_pool(name="ps_s", bufs=2, space="PSUM") as ps_s,
        tc.tile_pool(name="ps_pv", bufs=1, space="PSUM") as ps_pv,
        tc.tile_pool(name="ps_pj", bufs=2, space="PSUM") as ps_pj,
    ):
        for c in range(N_QCH):
            qcols = slice(c * RCH, (c + 1) * RCH)
            pt = p3.tile([P, NKT, RCH], F8, tag="pt")
            acc = p3.tile([P, RCH], F32, tag="acc")
            recip = rcp.tile([P, NSUB], F32, tag="recip")
            for kt in range(NKT):
                ps = ps_s.tile([P, RCH], F32, tag="ps_s")
                for pi in range(4):
                    nc.tensor.matmul(
                        ps,
                        kt_s[:, 2 * pi:2 * pi + 2, kt * P:(kt + 1) * P],
                        qt_s[:, 2 * pi:2 * pi + 2, qcols],
                        start=(pi == 0), stop=(pi == 3), perf_mode=DR)
                nc.scalar.activation(pt[:, kt, :], ps,
                                     mybir.ActivationFunctionType.Exp,
                                     scale=LN3 / 32.0)
                if kt == 0:
                    nc.vector.tensor_copy(acc, pt[:, 0, :])
                else:
                    nc.vector.tensor_tensor(acc, acc, pt[:, kt, :],
                                            mybir.AluOpType.add)
            if lvl < 3:
                continue
            # PV first: its first accumulation steps need only early pt
            # tiles, so the PE rolls straight from scores into PV while the
            # softmax-sum transposes wait for the last exp/acc to land
            ot = p3.tile([P, ND, RCH], F8, tag="ot")

            def pv_group(g):
                pvs = [ps_pv.tile([P, RCH], F32, tag=f"pv{j}",
                                  name=f"pv{c}_{g}_{j}")
                       for j in range(4)]
                for j in range(4):
                    for t2 in range(NKT // 2):
                        nc.tensor.matmul(
                            pvs[j],
                            v_s[:, 2 * t2:2 * t2 + 2,
                                g * 512 + j * P: g * 512 + (j + 1) * P],
                            pt[:, 2 * t2:2 * t2 + 2, :],
                            start=(t2 == 0), stop=(t2 == NKT // 2 - 1),
                            perf_mode=DR)
                    nc.scalar.mul(ot[:, g * 4 + j, :], pvs[j], OS)

            pv_group(0)
            for i in range(NSUB):
                pst = ps_s.tile([P, P], F32, tag="ps_s",
                                name=f"pstr{c}_{i}")
                nc.tensor.transpose(pst, acc[:, i * P:(i + 1) * P], ident)
                scol = rcp.tile([P, 1], F32, tag="scol")
                nc.vector.reduce_sum(scol, pst, axis=mybir.AxisListType.X)
                nc.vector.reciprocal(recip[:, i:i + 1], scol)
            pv_group(1)

            if lvl < 4:
                continue
            for qs in range(NSUB):
                for no in range(D // 512):
                    ps = ps_pj.tile([P, 512], F32, tag="pj")
                    for pi in range(4):
                        nc.tensor.matmul(
                            ps, ot[:, 2 * pi:2 * pi + 2, qs * P:(qs + 1) * P],
                            wp_sb[:, 2 * pi:2 * pi + 2, no * 512:(no + 1) * 512],
                            start=(pi == 0), stop=(pi == 3), perf_mode=DR)
                    o1 = outp.tile([P, 512], F32, tag="o1")
                    nc.vector.tensor_scalar_mul(o1, ps, recip[:, qs:qs + 1])
                    rt = outp.tile([P, 512], F32, tag="rt")
                    row0 = c * RCH + qs * P
                    # residual loads ride the scalar engine's DMA queue so
                    # they are not serialized behind the output stores
                    nc.scalar.dma_start(
                        rt, res_d[row0:row0 + P, no * 512:(no + 1) * 512])
                    o2 = outp.tile([P, 512], F32, tag="o2")
                    nc.vector.tensor_tensor(o2, o1, rt,
                                            mybir.AluOpType.add)
                    nc.sync.dma_start(
                        out_d[row0:row0 + P, no * 512:(no + 1) * 512], o2)


_CACHED = {}


def kernel(x, g_norm, w_qkv, b_qkv, w_proj, b_proj):
    global LAST_RESULT
    x = np.asarray(x, dtype=np.float32)
    g_norm = np.asarray(g_norm, dtype=np.float32)
    w_qkv = np.asarray(w_qkv, dtype=np.float32)
    b_qkv = np.asarray(b_qkv, dtype=np.float32)
    w_proj = np.asarray(w_proj, dtype=np.float32)
    b_proj = np.asarray(b_proj, dtype=np.float32)

    has_bqkv = bool(np.any(b_qkv))
    key = ("nc", has_bqkv)
    if key not in _CACHED:
        _CACHED[key] = _build(has_bqkv)
    nc = _CACHED[key]

    in_maps = _prepare_in_maps(x, g_norm, w_qkv, b_qkv, w_proj, b_proj)
    LAST_RESULT = run_bass_kernel_spmd(nc, in_maps, list(range(N_CORES)),
                                       trace=False)
    out = np.empty((B, S, D), dtype=np.float32)
    for c in range(N_CORES):
        b, h = c // 2, c % 2
        out[b, h * HALF:(h + 1) * HALF, :] = LAST_RESULT.results[c]["out"]
    return out
